# revision 6
# baseline (speedup 1.0000x reference)
"""Last-query sparse attention on 8 TRN2 NeuronCores.

Reference computation (per sample b):
    prev  = x[b, :-1, :]                 # [T-1, D]
    final = x[b, -1, :]                  # [D]
    s     = prev @ final                 # [T-1]
    w     = softmax(s)
    att   = w @ prev                     # [D]
    out   = concat(final, att)           # [2D]

Sharding: batch (B=64) split 8 ways -> 8 samples per core, no collectives.

Design (v2, derived from the v1 trace):
- The kernel is DMA-bound: 33.55MB f32 HBM read + 16.78MB fp16 SBUF write
  per core through 16 SDMA engines. v1 measured SDMA engine 15 ~15% slower
  than engines 0-14 (known SWDGE descriptor-ring contention), which alone
  stretched the load phase 94.5us -> 110us. Fix: non-uniform row assignment.
  Partition p holds a contiguous row range of x[b]; partitions served by
  engine 15 ({92..95, 124..127}) get 28 rows, partitions 0..31 get 33, the
  rest 32 (total 4096). The unused (p, i) score slots are zero-padded once
  (zero products -> score 0 -> weight exp(0-gmax) ~ e^-55 ~ 0).
- All 8 samples' loads are issued up front into 8 persistent SBUF tiles
  (16.9KB/partition each) as SWDGE f32->fp16 cast DMAs; nothing else runs
  on the GpSimd queue, so descriptors stream uninterrupted. The last
  sample's load is split in two so its pass-1 overlaps the DMA tail.
- Pass 1 on DVE per 17-block chunk: fp16 product vs broadcast query, two
  pairwise tree-add levels, segmented f32 reduce -> S [128, 33].
- Softmax with no GpSimd: row max (DVE) -> transpose to one partition via
  a 1-column matmul against an identity (PE) -> max over the row (DVE) ->
  broadcast -gmax to all partitions via a negated-ones matmul (PE) -> ACT
  exp with the PSUM-copied bias. The denominator comes from a ones-vector
  matmul over the exp weights (PE) + row reduce (DVE).
- Pass 2: 33 matmuls (lhsT = weight column [128,1], rhs = fp16 X block
  [128,256]) alternating between two PSUM banks to avoid same-bank
  accumulate stalls; combined + normalized per sample and stored.

Measured v1 baseline: 131.1us. HBM/SDMA floor ~93us + ~7us preamble.
"""

import sys

sys.path.insert(0, "/opt/trn_rl_repo")

from contextlib import ExitStack

import numpy as np

import concourse.tile as tile
from concourse import bacc, mybir
from concourse.bass_utils import run_bass_kernel_spmd

N_CORES = 8
B = 64
T = 4096
D = 256
BPC = B // N_CORES  # samples per core
P = 128
NBLK = 33  # padded block count; t rows are distributed non-uniformly
F32 = mybir.dt.float32
FP16 = mybir.dt.float16

# (p0, p1, rows, row_offset): partition range [p0,p1) holds `rows`
# contiguous rows of x[b] starting at row_offset + (p-p0)*rows.
# Engine-15 partitions ({92..95},{124..127}) get 28 rows; 0..31 get 33.
RANGES = [
    (0, 32, 33, 0),
    (32, 92, 32, 1056),
    (92, 96, 28, 2976),
    (96, 124, 32, 3088),
    (124, 128, 28, 3984),
]
# self-score: row 4095 lives at partition 127, block 27
MASK_COL = 27
# pass-1 chunks along the block dim
CHUNKS = [(0, 17), (17, 33)]

_NC_CACHE = None


def _build():
    nc = bacc.Bacc(
        trn_type="TRN2",
        target_bir_lowering=False,
        debug=False,
        num_devices=N_CORES,
    )
    x_ext = nc.declare_dram_parameter("x", [BPC, T, D], F32, isOutput=False)
    ident_ext = nc.declare_dram_parameter("cst_ident", [P, P], FP16, isOutput=False)
    ones_ext = nc.declare_dram_parameter("cst_ones", [P, 1], FP16, isOutput=False)
    nones_ext = nc.declare_dram_parameter("cst_negones", [1, P], FP16, isOutput=False)
    mask_ext = nc.declare_dram_parameter("cst_mask", [P, 1], F32, isOutput=False)
    out_ext = nc.declare_dram_parameter("out", [BPC, 2 * D], F32, isOutput=True)
    xap = x_ext.ap()
    oap = out_ext.ap()

    with ExitStack() as ctx:
        tc = ctx.enter_context(tile.TileContext(nc))
        xpool = ctx.enter_context(tc.tile_pool(name="xp", bufs=8))
        fpool = ctx.enter_context(tc.tile_pool(name="fp", bufs=8))
        fhpool = ctx.enter_context(tc.tile_pool(name="fhp", bufs=8))
        scr = ctx.enter_context(tc.tile_pool(name="scr", bufs=2))
        spool = ctx.enter_context(tc.tile_pool(name="sp", bufs=2))
        pwpool = ctx.enter_context(tc.tile_pool(name="pw", bufs=2))
        stat = ctx.enter_context(tc.tile_pool(name="stat", bufs=2))
        cpool = ctx.enter_context(tc.tile_pool(name="const", bufs=1))
        psa = ctx.enter_context(tc.tile_pool(name="psa", bufs=2, space="PSUM"))
        psb = ctx.enter_context(tc.tile_pool(name="psb", bufs=2, space="PSUM"))
        psaux = ctx.enter_context(tc.tile_pool(name="psx", bufs=2, space="PSUM"))
        psn = ctx.enter_context(tc.tile_pool(name="psn", bufs=2, space="PSUM"))

        # constants from host (loaded on the HWDGE queue, off the load path)
        ident16 = cpool.tile([P, P], FP16)
        nc.sync.dma_start(ident16[:], ident_ext.ap())
        ones16 = cpool.tile([P, 1], FP16)
        nc.sync.dma_start(ones16[:], ones_ext.ap())
        negones16 = cpool.tile([1, P], FP16)
        nc.sync.dma_start(negones16[:], nones_ext.ap())
        maskbias = cpool.tile([P, 1], F32)
        nc.sync.dma_start(maskbias[:], mask_ext.ap())

        # 8 persistent fp16 X tiles; pad slots zeroed once (DVE), never
        # overwritten by the DMAs.
        xtiles = [
            xpool.tile([P, NBLK, D], FP16, tag="xh", name=f"xh{b}") for b in range(BPC)
        ]
        # DVE partition-offset ops need 32-aligned, 32-wide windows; zeroing
        # a few real-data slots is fine -- the loads overwrite them.
        for xt in xtiles:
            nc.vector.memset(xt[32:64, 32:NBLK, :], 0.0)
            nc.vector.memset(xt[64:96, 28:NBLK, :], 0.0)
            nc.vector.memset(xt[96:128, 28:NBLK, :], 0.0)

        # ---- issue every load up front ----
        ftiles, fhtiles = [], []
        for b in range(BPC):
            xt = xtiles[b]
            # last sample: split at block 17 so its pass-1 overlaps the tail
            csplits = CHUNKS if b == BPC - 1 else [(0, NBLK)]
            for c0, c1 in csplits:
                for p0, p1, rows, off in RANGES:
                    r0, r1 = min(c0, rows), min(c1, rows)
                    if r1 <= r0:
                        continue
                    src = xap[b, off : off + (p1 - p0) * rows].rearrange(
                        "(p i) d -> p i d", p=p1 - p0
                    )[:, r0:r1, :]
                    nc.gpsimd.dma_start(xt[p0:p1, r0:r1, :], src)
            F = fpool.tile([P, D], F32)
            nc.sync.dma_start(F[:], xap[b, T - 1].partition_broadcast(P))
            # final row goes out as-is
            nc.sync.dma_start(oap[b : b + 1, 0:D], F[0:1, :])
            Fh = fhpool.tile([P, D], FP16)
            nc.scalar.copy(Fh[:], F[:])
            ftiles.append(F)
            fhtiles.append(Fh)

        # ---- per-sample compute ----
        for b in range(BPC):
            xt = xtiles[b]
            Fh = fhtiles[b]

            S = spool.tile([P, NBLK], F32)
            for c0, c1 in CHUNKS:
                cn = c1 - c0
                prod = scr.tile([P, 17, D], FP16, tag="prod")
                nc.vector.tensor_mul(
                    prod[:, 0:cn, :],
                    xt[:, c0:c1, :],
                    Fh[:].unsqueeze(1).broadcast_to((P, cn, D)),
                )
                l1 = scr.tile([P, 17, D // 2], FP16, tag="l1")
                nc.vector.tensor_add(
                    l1[:, 0:cn, :],
                    prod[:, 0:cn, 0 : D // 2],
                    prod[:, 0:cn, D // 2 : D],
                )
                l2 = scr.tile([P, 17, D // 4], FP16, tag="l2")
                nc.vector.tensor_add(
                    l2[:, 0:cn, :],
                    l1[:, 0:cn, 0 : D // 4],
                    l1[:, 0:cn, D // 4 : D // 2],
                )
                nc.vector.reduce_sum(
                    S[:, c0:c1], l2[:, 0:cn, :], axis=mybir.AxisListType.X
                )
            # mask the query's self-score (p=127, block 27)
            nc.vector.tensor_add(
                S[:, MASK_COL : MASK_COL + 1], S[:, MASK_COL : MASK_COL + 1], maskbias[:]
            )

            # global max via PE transpose + DVE, broadcast back negated via PE
            rowmax16 = stat.tile([P, 1], FP16, tag="rm")
            nc.vector.reduce_max(rowmax16[:], S[:], axis=mybir.AxisListType.X)
            psT = psaux.tile([1, P], F32, tag="aux")
            nc.tensor.matmul(psT[:], lhsT=rowmax16[:], rhs=ident16[:], start=True, stop=True)
            gmax16 = stat.tile([1, 1], FP16, tag="gm")
            nc.vector.reduce_max(gmax16[:], psT[:], axis=mybir.AxisListType.X)
            psN = psn.tile([P, 1], F32, tag="ng")
            nc.tensor.matmul(psN[:], lhsT=negones16[:], rhs=gmax16[:], start=True, stop=True)
            negmax = stat.tile([P, 1], F32, tag="nm")
            nc.scalar.copy(negmax[:], psN[:])

            Pw = pwpool.tile([P, NBLK], FP16)
            nc.scalar.activation(
                Pw[:],
                S[:],
                mybir.ActivationFunctionType.Exp,
                bias=negmax[:],
                scale=1.0,
            )

            # pass 2: alternate PSUM banks to dodge same-bank accumulate stalls
            pA = psa.tile([1, D], F32, tag="pa")
            pB = psb.tile([1, D], F32, tag="pb")
            for i in range(NBLK):
                ps = pA if i % 2 == 0 else pB
                nc.tensor.matmul(
                    ps[:],
                    lhsT=Pw[:, i : i + 1],
                    rhs=xt[:, i, :],
                    start=(i < 2),
                    stop=(i >= NBLK - 2),
                )
            # denominator: ones^T @ Pw -> column sums -> row reduce
            psZ = psaux.tile([1, P], F32, tag="aux")
            nc.tensor.matmul(psZ[:, 0:NBLK], lhsT=ones16[:], rhs=Pw[:], start=True, stop=True)
            z = stat.tile([1, 1], F32, tag="z")
            nc.vector.reduce_sum(z[:], psZ[:, 0:NBLK], axis=mybir.AxisListType.X)
            rz = stat.tile([1, 1], F32, tag="rz")
            nc.vector.reciprocal(rz[:], z[:])

            att_b = stat.tile([1, D], F32, tag="ab")
            nc.scalar.copy(att_b[:], pB[:])
            att_u = stat.tile([1, D], F32, tag="au")
            nc.vector.tensor_add(att_u[:], pA[:], att_b[:])
            att_n = stat.tile([1, D], F32, tag="an")
            nc.vector.tensor_mul(att_n[:], att_u[:], rz[:].broadcast_to((1, D)))
            nc.sync.dma_start(oap[b : b + 1, D : 2 * D], att_n[:])

    nc.compile()
    return nc


def _consts():
    return {
        "cst_ident": np.eye(P, dtype=np.float16),
        "cst_ones": np.ones((P, 1), dtype=np.float16),
        "cst_negones": np.full((1, P), -1.0, dtype=np.float16),
        "cst_mask": np.concatenate(
            [np.zeros((P - 1, 1), np.float32), np.full((1, 1), -1.0e30, np.float32)]
        ),
    }


def _run(x, trace=False):
    global _NC_CACHE
    x = np.ascontiguousarray(np.asarray(x, dtype=np.float32))
    assert x.shape == (B, T, D), x.shape
    if _NC_CACHE is None:
        _NC_CACHE = _build()
    cst = _consts()
    in_maps = [
        {"x": x[c * BPC : (c + 1) * BPC], **cst} for c in range(N_CORES)
    ]
    res = run_bass_kernel_spmd(
        _NC_CACHE, in_maps, core_ids=list(range(N_CORES)), trace=trace
    )
    out = np.concatenate([res.results[c]["out"] for c in range(N_CORES)], axis=0)
    return out.astype(np.float32), res


def kernel(x):
    out, _ = _run(x, trace=False)
    return out


# revision 13
# speedup vs baseline: 1.5884x; 1.5884x over previous
"""Last-query sparse attention on 8 TRN2 NeuronCores.

Reference computation (per sample b):
    prev  = x[b, :-1, :]                 # [T-1, D]
    final = x[b, -1, :]                  # [D]
    s     = prev @ final                 # [T-1]
    w     = softmax(s)
    att   = w @ prev                     # [D]
    out   = concat(final, att)           # [2D]

Sharding: batch (B=64) split 8 ways -> 8 samples per core, no collectives.

Design notes (v3):
- DMA-bound kernel: 33.55MB f32 HBM read + 16.78MB fp16 SBUF write per
  core. SDMA engine 15 is ~15% slower than engines 0-14 (SWDGE ring
  contention), so rows are assigned non-uniformly: partitions served by
  engine 15 ({92..95, 124..127}) hold 28 rows of x[b], partitions 0..31
  hold 33, the rest 32 (total 4096). Unused (p, i) slots are zeroed once
  (zero products -> score 0 -> weight ~ e^-55 ~ 0 since gmax ~ 55).
- All loads are issued up front into 8 persistent fp16 tiles (SWDGE cast
  DMAs, the only work on the GpSimd queue besides 8 interleaved
  partition_all_reduce calls). The last sample loads in 4 chunks so its
  pass-1 rides the DMA tail.
- Engine queues are strict FIFO, so the loop is software-pipelined:
  sample b's epilogue (denominator reduce + normalize) is issued after
  sample b+1's pass-1 so it never head-blocks the Vector queue, and the
  PE queue carries only the pass-2 matmul stream (+ one denominator
  matmul per sample), keeping HAM warm.
- Softmax: row max (DVE) -> cross-partition max on GpSimd -> negate
  (ACT) -> exp (ACT). Denominator: ones^T @ Pw on PE + row reduce.
- Pass 2: 33 matmuls (lhsT = weight column [128,1], rhs = fp16 X block
  [128,256]) alternating between two PSUM banks; att = bankA + bankB
  (via one ACT PSUM->SBUF copy), scaled by 1/Z, stored per sample.
"""

import sys

sys.path.insert(0, "/opt/trn_rl_repo")

from contextlib import ExitStack

import numpy as np

import concourse.tile as tile
import concourse.bass_isa as bass_isa
from concourse import bacc, mybir
from concourse.bass_utils import run_bass_kernel_spmd

N_CORES = 8
B = 64
T = 4096
D = 256
BPC = B // N_CORES  # samples per core
P = 128
NBLK = 33  # padded block count; t rows are distributed non-uniformly
F32 = mybir.dt.float32
FP16 = mybir.dt.float16

# (p0, p1, rows, row_offset): partition range [p0,p1) holds `rows`
# contiguous rows of x[b] starting at row_offset + (p-p0)*rows.
RANGES = [
    (0, 32, 33, 0),
    (32, 92, 32, 1056),
    (92, 96, 28, 2976),
    (96, 124, 32, 3088),
    (124, 128, 28, 3984),
]
MASK_COL = 27  # self-score: row 4095 lives at partition 127, block 27
CHUNKS = [(0, 17), (17, 33)]  # pass-1 chunks
CHUNKS_LAST = [(0, 9), (9, 17), (17, 25), (25, 33)]

_NC_CACHE = None


def _build():
    nc = bacc.Bacc(
        trn_type="TRN2",
        target_bir_lowering=False,
        debug=False,
        num_devices=N_CORES,
    )
    x_ext = nc.declare_dram_parameter("x", [BPC, T, D], F32, isOutput=False)
    ones_ext = nc.declare_dram_parameter("cst_ones", [P, 1], FP16, isOutput=False)
    mask_ext = nc.declare_dram_parameter("cst_mask", [P, 1], F32, isOutput=False)
    zero_ext = nc.declare_dram_parameter("cst_zeros", [4, 5, D], FP16, isOutput=False)
    out_ext = nc.declare_dram_parameter("out", [BPC, 2 * D], F32, isOutput=True)
    xap = x_ext.ap()
    oap = out_ext.ap()

    with ExitStack() as ctx:
        tc = ctx.enter_context(tile.TileContext(nc))
        xpool = ctx.enter_context(tc.tile_pool(name="xp", bufs=8))
        fpool = ctx.enter_context(tc.tile_pool(name="fp", bufs=8))
        fhpool = ctx.enter_context(tc.tile_pool(name="fhp", bufs=8))
        scr = ctx.enter_context(tc.tile_pool(name="scr", bufs=3))
        spool = ctx.enter_context(tc.tile_pool(name="sp", bufs=2))
        pwpool = ctx.enter_context(tc.tile_pool(name="pw", bufs=2))
        rpool = ctx.enter_context(tc.tile_pool(name="rp", bufs=8))
        stat = ctx.enter_context(tc.tile_pool(name="stat", bufs=2))
        cpool = ctx.enter_context(tc.tile_pool(name="const", bufs=1))
        psa = ctx.enter_context(tc.tile_pool(name="psa", bufs=2, space="PSUM"))
        psb = ctx.enter_context(tc.tile_pool(name="psb", bufs=2, space="PSUM"))
        psz = ctx.enter_context(tc.tile_pool(name="psz", bufs=2, space="PSUM"))

        ones16 = cpool.tile([P, 1], FP16)
        nc.sync.dma_start(ones16[:], ones_ext.ap())
        maskbias = cpool.tile([P, 1], F32)
        nc.sync.dma_start(maskbias[:], mask_ext.ap())

        xtiles = [
            xpool.tile([P, NBLK, D], FP16, tag="xh", name=f"xh{b}") for b in range(BPC)
        ]
        # cross-partition max results, written by gpsimd ARs issued early
        rmx = [rpool.tile([P, 1], F32, tag="rm", name=f"rm{b}") for b in range(BPC)]
        gmx = [rpool.tile([P, 1], F32, tag="gm", name=f"gm{b}") for b in range(BPC)]

        # ---- pad init + load issue (called inline from the compute loop) ----
        ftiles, fhtiles = [], []

        def init_and_load(b):
            xt = xtiles[b]
            # zero the pad slots: 32-aligned slivers on DVE, the unaligned
            # engine-15 ranges via tiny host-constant DMAs (HWDGE)
            nc.vector.memset(xt[32:64, 32:NBLK, :], 0.0)
            nc.vector.memset(xt[64:96, 32:NBLK, :], 0.0)
            nc.vector.memset(xt[96:124, 32:NBLK, :], 0.0)
            nc.sync.dma_start(xt[92:96, 28:NBLK, :], zero_ext.ap())
            nc.sync.dma_start(xt[124:128, 28:NBLK, :], zero_ext.ap())

            csplits = CHUNKS_LAST if b == BPC - 1 else [(0, NBLK)]
            for c0, c1 in csplits:
                for p0, p1, rows, off in RANGES:
                    r0, r1 = min(c0, rows), min(c1, rows)
                    if r1 <= r0:
                        continue
                    src = xap[b, off : off + (p1 - p0) * rows].rearrange(
                        "(p i) d -> p i d", p=p1 - p0
                    )[:, r0:r1, :]
                    nc.gpsimd.dma_start(xt[p0:p1, r0:r1, :], src)
            F = fpool.tile([P, D], F32, tag="f", name=f"f{b}")
            nc.sync.dma_start(F[:], xap[b, T - 1].partition_broadcast(P))
            nc.sync.dma_start(oap[b : b + 1, 0:D], F[0:1, :])
            Fh = fhpool.tile([P, D], FP16, tag="fh", name=f"fh{b}")
            nc.scalar.copy(Fh[:], F[:])
            ftiles.append(F)
            fhtiles.append(Fh)

        # ---- software-pipelined compute ----
        # stage A(b): pass-1 + row max (DVE) ; then ACT: negate+exp ; PE: pass-2
        # stage B(b): denominator reduce + normalize + store, issued after
        #             A(b+1) so it never head-blocks the Vector queue.
        pend = {}  # b -> (pA, pB, psZ_slice)

        def stage_a(b):
            xt = xtiles[b]
            Fh = fhtiles[b]
            S = spool.tile([P, NBLK], F32, tag="s", name=f"s{b}")
            chunks = CHUNKS_LAST if b == BPC - 1 else CHUNKS
            for c0, c1 in chunks:
                cn = c1 - c0
                prod = scr.tile([P, 17, D], FP16, tag="prod", name=f"pr{b}_{c0}")
                nc.vector.tensor_mul(
                    prod[:, 0:cn, :],
                    xt[:, c0:c1, :],
                    Fh[:].unsqueeze(1).broadcast_to((P, cn, D)),
                )
                l1 = scr.tile([P, 17, D // 2], FP16, tag="l1", name=f"l1_{b}_{c0}")
                nc.vector.tensor_add(
                    l1[:, 0:cn, :],
                    prod[:, 0:cn, 0 : D // 2],
                    prod[:, 0:cn, D // 2 : D],
                )
                l2 = scr.tile([P, 17, D // 4], FP16, tag="l2", name=f"l2_{b}_{c0}")
                nc.vector.tensor_add(
                    l2[:, 0:cn, :],
                    l1[:, 0:cn, 0 : D // 4],
                    l1[:, 0:cn, D // 4 : D // 2],
                )
                nc.vector.reduce_sum(
                    S[:, c0:c1], l2[:, 0:cn, :], axis=mybir.AxisListType.X
                )
            nc.vector.tensor_add(
                S[:, MASK_COL : MASK_COL + 1],
                S[:, MASK_COL : MASK_COL + 1],
                maskbias[:],
            )
            nc.vector.reduce_max(rmx[b][:], S[:], axis=mybir.AxisListType.X)
            # cross-partition max on GpSimd (only waits block later ARs,
            # never DMA issues: loads for b+4 were issued a step earlier)
            nc.gpsimd.partition_all_reduce(
                gmx[b][:], rmx[b][:], channels=P, reduce_op=bass_isa.ReduceOp.max
            )

            # ACT: negate the gpsimd-reduced max, exponentiate
            negmax = stat.tile([P, 1], F32, tag="nm", name=f"nm{b}")
            nc.scalar.mul(negmax[:], gmx[b][:], -1.0)
            Pw = pwpool.tile([P, NBLK], FP16, tag="pw", name=f"pw{b}")
            nc.scalar.activation(
                Pw[:],
                S[:],
                mybir.ActivationFunctionType.Exp,
                bias=negmax[:],
                scale=1.0,
            )

            # PE: pass-2 stream, alternating PSUM banks
            pA = psa.tile([1, D], F32, tag="pa", name=f"pa{b}")
            pB = psb.tile([1, D], F32, tag="pb", name=f"pb{b}")
            for i in range(NBLK):
                ps = pA if i % 2 == 0 else pB
                nc.tensor.matmul(
                    ps[:],
                    lhsT=Pw[:, i : i + 1],
                    rhs=xt[:, i, :],
                    start=(i < 2),
                    stop=(i >= NBLK - 2),
                )
            pZ = psz.tile([1, P], F32, tag="pz", name=f"pz{b}")
            nc.tensor.matmul(
                pZ[:, 0:NBLK], lhsT=ones16[:], rhs=Pw[:], start=True, stop=True
            )
            pend[b] = (pA, pB, pZ)

        def stage_b(b):
            pA, pB, pZ = pend.pop(b)
            att_b = stat.tile([1, D], F32, tag="ab", name=f"ab{b}")
            nc.scalar.copy(att_b[:], pB[:])  # ACT drains bank B
            z = stat.tile([1, 1], F32, tag="z", name=f"z{b}")
            nc.vector.reduce_sum(z[:], pZ[:, 0:NBLK], axis=mybir.AxisListType.X)
            rz = stat.tile([1, 1], F32, tag="rz", name=f"rz{b}")
            nc.vector.reciprocal(rz[:], z[:])
            att_u = stat.tile([1, D], F32, tag="au", name=f"au{b}")
            nc.vector.tensor_add(att_u[:], pA[:], att_b[:])
            att_n = stat.tile([1, D], F32, tag="an", name=f"an{b}")
            nc.vector.tensor_mul(att_n[:], att_u[:], rz[:].broadcast_to((1, D)))
            nc.sync.dma_start(oap[b : b + 1, D : 2 * D], att_n[:])

        for b in range(4):
            init_and_load(b)
        for b in range(BPC):
            if b + 4 < BPC:
                init_and_load(b + 4)
            stage_a(b)
            if b > 0:
                stage_b(b - 1)
        stage_b(BPC - 1)

    nc.compile()
    return nc


def _consts():
    return {
        "cst_ones": np.ones((P, 1), dtype=np.float16),
        "cst_mask": np.concatenate(
            [np.zeros((P - 1, 1), np.float32), np.full((1, 1), -1.0e30, np.float32)]
        ),
        "cst_zeros": np.zeros((4, 5, D), dtype=np.float16),
    }


def _run(x, trace=False):
    global _NC_CACHE
    x = np.ascontiguousarray(np.asarray(x, dtype=np.float32))
    assert x.shape == (B, T, D), x.shape
    if _NC_CACHE is None:
        _NC_CACHE = _build()
    cst = _consts()
    in_maps = [{"x": x[c * BPC : (c + 1) * BPC], **cst} for c in range(N_CORES)]
    res = run_bass_kernel_spmd(
        _NC_CACHE, in_maps, core_ids=list(range(N_CORES)), trace=trace
    )
    out = np.concatenate([res.results[c]["out"] for c in range(N_CORES)], axis=0)
    return out.astype(np.float32), res


def kernel(x):
    out, _ = _run(x, trace=False)
    return out
